# revision 57
# baseline (speedup 1.0000x reference)
"""BigramHash embedding lookup kernel for 8 Trainium2 NeuronCores.

Strategy (row-range-sharded table via quantile split, host-side routing
since we receive full inputs):
  - Host computes bucket ids h = (prev_id * MULT + id) % NUM_BUCKETS.
  - Tokens are sorted by bucket id and split into 8 EQUAL groups of T/8
    tokens (quantile split), so every core processes exactly K = T/8/128
    blocks — no padding from load imbalance.  Each core receives the
    contiguous table row range its tokens touch, padded to a fixed
    SHARD_CAP so the compiled program is input-independent.  Rows are
    stored bf16, 128 wide with the 64 hash dims DUPLICATED, so a
    transposed gather lands the hash dims on PSUM-row groups 0-63 AND
    64-127 (both PE weight tiles usable).
  - The first PRE blocks are pre-gathered (transposed) ON THE HOST into a
    dense emb0T input, so the device pipeline starts on a plain DMA load
    instead of the idx-load -> gather latency chain.
  - Remaining blocks arrive via gpsimd dma_gather(transpose=True): 256
    sorted tokens per call from a static 32768-row window whose base
    tracks the sorted-position quantiles (int16 indices; feasibility is
    verified on the host, with a fallback program if violated).  The
    gather itself delivers [hash_dim, tokens] — no PE transpose and no
    PSUM round-trip for the operands at all.
  - Matmuls project 128-token blocks to model dim (bf16 operands, f32
    PSUM), the result is cast to bf16 by DVE/Act copies and stored as a
    [CAP, 1024] bf16 slab batched 4 blocks per DMA on the SP ring (the
    f32 upcast happens on the host).
  - Host scatters the per-core slabs back to the original token order.

All heavy HBM traffic is bf16; the output stream (2 KiB/token/core at
~360 GB/s) is the roofline.
"""

from contextlib import ExitStack

import ml_dtypes
import numpy as np

import concourse.bass as bass
import concourse.mybir as mybir
import concourse.tile as tile
from concourse import bacc
from concourse.bass import IndirectOffsetOnAxis
from concourse.bass_utils import run_bass_kernel_spmd
from concourse.masks import make_identity

NUM_BUCKETS = 2000003
HASH_DIM = 64
MODEL_DIM = 1024
HASH_MULT = 92821
N_CORES = 8
P = 128
SHARD_CAP = 299008  # fixed per-core table rows (~1.2x the uniform share)
NFREE = 512  # matmul moving-operand free dim (one PSUM bank of f32)
CB = 3  # blocks per dma_gather call (384 tokens)
W = 32768  # dma_gather window rows (int16 index range)
PRE_MAX = 8  # host-pre-gathered leading blocks
R0 = NUM_BUCKETS // N_CORES  # expected per-core row range

_prog_cache: dict = {}


def _plan(K: int):
    pre = min(PRE_MAX, K)
    cap = K * P
    # blocks per gather sized so the expected sorted-row span of a chunk
    # stays well under the int16 window W
    cb = max(1, min(CB, (cap * 24576) // (R0 * P) if R0 else CB))
    chunks = []  # (first block, #blocks) per dma_gather call
    b = pre
    while b < K:
        nb = min(cb, K - b)
        chunks.append((b, nb))
        b += nb
    bounds = [1, min(2, K), min(4, K)] if K > 1 else [1]
    bounds = sorted(set(bounds))
    while bounds[-1] < K:
        bounds.append(min(bounds[-1] + 4, K))
    return pre, chunks, bounds


def _common_pools(nc, tc, ctx, n_emb):
    const_p = ctx.enter_context(tc.tile_pool(name="const", bufs=1))
    idx_p = ctx.enter_context(tc.tile_pool(name="idx", bufs=1))
    emb_p = ctx.enter_context(tc.tile_pool(name="emb", bufs=max(n_emb, 1)))
    out_p = ctx.enter_context(tc.tile_pool(name="out", bufs=4))
    return const_p, idx_p, emb_p, out_p


def _build_program(K: int) -> "bacc.Bacc":
    """Fast path: transposed dma_gather from quantile windows."""
    nc = bacc.Bacc(
        "TRN2",
        target_bir_lowering=False,
        debug=False,
        num_devices=N_CORES,
        dynamic_dma_scratch_size=65536,
    )
    f32 = mybir.dt.float32
    bf16 = mybir.dt.bfloat16
    pre, chunks, bounds = _plan(K)

    n_idx_cols = sum(nb * P // 16 for _b, nb in chunks)
    # boot = [pre-gathered transposed blocks | projT] in one upload: a single
    # HWDGE slot puts the whole first-store dependency chain in flight at t=0
    boot_d = nc.dram_tensor(
        "boot", [HASH_DIM, pre * P + MODEL_DIM], bf16, kind="ExternalInput"
    ).ap()
    if chunks:
        idx_d = nc.dram_tensor(
            "idx16", [P, n_idx_cols], mybir.dt.int16, kind="ExternalInput"
        ).ap()
        # per-chunk window slices, concatenated: chunk ci gathers from rows
        # [ci*W, (ci+1)*W) — windows are exact by construction
        tab_d = nc.dram_tensor(
            "table", [len(chunks) * W, P], bf16, kind="ExternalInput"
        ).ap()
    projhi_d = nc.dram_tensor(
        "projhi", [HASH_DIM, MODEL_DIM], bf16, kind="ExternalInput"
    ).ap()
    out_d = nc.dram_tensor("out", [P * K, MODEL_DIM], bf16, kind="ExternalOutput").ap()

    with tile.TileContext(nc) as tc, ExitStack() as ctx:
        const_p, idx_p, emb_p, out_p = _common_pools(nc, tc, ctx, len(chunks))
        ps_mm = ctx.enter_context(tc.tile_pool(name="ps_mm", bufs=7, space="PSUM"))
        ps_w = ctx.enter_context(tc.tile_pool(name="ps_w", bufs=1, space="PSUM"))

        # SP ring: boot first, split as [2 blocks | projT] + [rest] so the
        # first store's whole dependency chain rides one early DMA; then all
        # stores.  Act ring: idx16 + projhi.
        boot_t = const_p.tile([HASH_DIM, pre * P + MODEL_DIM], bf16)
        cut = min(2, pre) * P + MODEL_DIM
        nc.sync.dma_start(out=boot_t[:, :cut], in_=boot_d[:, :cut])
        if pre * P + MODEL_DIM > cut:
            nc.sync.dma_start(out=boot_t[:, cut:], in_=boot_d[:, cut:])
        if chunks:
            idx_t = idx_p.tile([P, n_idx_cols], mybir.dt.int16)
            nc.scalar.dma_start(out=idx_t[:], in_=idx_d[:])

        # PE warm-up: ramps the PE clock to full p-state and keeps it busy
        # until the pre-gathered blocks land
        dumw = const_p.tile([P, NFREE], bf16)
        nc.vector.memset(dumw[:], 0.0)
        wps = ps_w.tile([P, NFREE], f32)
        for _ in range(4):
            nc.tensor.matmul(
                wps[:], lhsT=dumw[:, :P], rhs=dumw[:], start=True, stop=True
            )

        # projT lives twice: in boot on partitions 0-63 (even blocks) and
        # in projhi_t on partitions 64-127 (odd blocks) so the two PE row
        # groups double-buffer weights
        projhi_t = const_p.tile([P, MODEL_DIM], bf16)
        nc.scalar.dma_start(out=projhi_t[HASH_DIM:, :], in_=projhi_d[:])

        # all gathers up-front; each delivers [128 elem, nb*128 tokens] with
        # the hash dims already on partitions (no PE transpose needed)
        embTs = {}
        icol = 0
        for ci, (b0, nb) in enumerate(chunks):
            ct = nb * P
            embT = emb_p.tile([P, ct], bf16, name=f"embT{b0}")
            nc.gpsimd.dma_gather(
                embT[:].rearrange("p (c t) -> p c t", t=ct),
                tab_d[ci * W : (ci + 1) * W, :],
                idx_t[:, icol : icol + ct // 16],
                ct,
                ct,
                P,
                transpose=True,
            )
            embTs[b0] = embT
            icol += ct // 16

        sb_of = {}  # block -> (store span start, span size)
        for lo, hi in zip([0] + bounds[:-1], bounds):
            for b in range(lo, hi):
                sb_of[b] = (lo, hi - lo)

        hc = 0
        state = {"o_t": None}

        def emit_block(b, src, scol, dual=True):
            """matmuls + copies (+ store flush) for 128-token block b; the
            block's tokens are columns [scol, scol+128) of src."""
            nonlocal hc
            q = (b % 2) * HASH_DIM if dual else 0
            sb0, sbn = sb_of[b]
            if state["o_t"] is None:
                state["o_t"] = out_p.tile([P, sbn * MODEL_DIM], bf16, name="o_t")
            o_t = state["o_t"]
            jo = b - sb0
            for n in range(MODEL_DIM // NFREE):
                mm = ps_mm.tile([P, NFREE], f32)
                pj = min(2, pre) * P
                rhs = (
                    boot_t[:, pj + n * NFREE : pj + (n + 1) * NFREE]
                    if q == 0
                    else projhi_t[HASH_DIM:, n * NFREE : (n + 1) * NFREE]
                )
                nc.tensor.matmul(
                    mm[:],
                    lhsT=src[q : q + HASH_DIM, scol : scol + P],
                    rhs=rhs,
                    start=True,
                    stop=True,
                )
                dst = o_t[
                    :, jo * MODEL_DIM + n * NFREE : jo * MODEL_DIM + (n + 1) * NFREE
                ]
                if hc % 2 == 0:
                    nc.vector.tensor_copy(dst, mm[:])
                else:
                    nc.scalar.copy(dst, mm[:])
                hc += 1
            if b + 1 == sb0 + sbn:
                dst_ap = out_d[sb0 * P : (sb0 + sbn) * P, :].rearrange(
                    "(b p) m -> p b m", p=P
                )
                src_ap = o_t[:].rearrange("p (b m) -> p b m", m=MODEL_DIM)
                nc.sync.dma_start(out=dst_ap, in_=src_ap)
                state["o_t"] = None

        for b in range(pre):
            scol = b * P if b < 2 else MODEL_DIM + b * P
            emit_block(b, boot_t, scol, dual=False)
        for b0, nb in chunks:
            embT = embTs[b0]
            for j in range(nb):
                emit_block(b0 + j, embT, j * P)
    nc.compile()
    return nc


def _build_program_ind(K: int) -> "bacc.Bacc":
    """Fallback: per-block single-offset indirect gathers + PE transposes.
    Used only when the quantile windows are infeasible (pathological
    inputs); correct for any int32 row ids < SHARD_CAP."""
    nc = bacc.Bacc(
        "TRN2",
        target_bir_lowering=False,
        debug=False,
        num_devices=N_CORES,
        dynamic_dma_scratch_size=65536,
    )
    f32 = mybir.dt.float32
    bf16 = mybir.dt.bfloat16
    idx_d = nc.dram_tensor("idx", [P, K], mybir.dt.int32, kind="ExternalInput").ap()
    tab_d = nc.dram_tensor(
        "table", [SHARD_CAP, HASH_DIM], bf16, kind="ExternalInput"
    ).ap()
    projT_d = nc.dram_tensor(
        "projT", [HASH_DIM, MODEL_DIM], bf16, kind="ExternalInput"
    ).ap()
    out_d = nc.dram_tensor("out", [P * K, MODEL_DIM], bf16, kind="ExternalOutput").ap()

    with tile.TileContext(nc) as tc, ExitStack() as ctx:
        const_p, idx_p, emb_p, out_p = _common_pools(nc, tc, ctx, 4)
        embT_p = ctx.enter_context(tc.tile_pool(name="embT", bufs=3))
        ps_t = ctx.enter_context(tc.tile_pool(name="ps_t", bufs=2, space="PSUM"))
        ps_mm = ctx.enter_context(tc.tile_pool(name="ps_mm", bufs=5, space="PSUM"))

        ident = const_p.tile([P, P], bf16)
        make_identity(nc, ident[:])
        projT_s = const_p.tile([P, MODEL_DIM], bf16)
        nc.scalar.dma_start(out=projT_s[:HASH_DIM, :], in_=projT_d[:])
        nc.scalar.dma_start(out=projT_s[HASH_DIM:, :], in_=projT_d[:])
        idx_t = idx_p.tile([P, K], mybir.dt.int32)
        nc.sync.dma_start(out=idx_t[:], in_=idx_d[:])

        for pb in range(0, K, 2):
            nblocks = min(2, K - pb)
            embp = emb_p.tile([P, nblocks * HASH_DIM], bf16, name="embp")
            for j in range(nblocks):
                nc.gpsimd.indirect_dma_start(
                    out=embp[:, j * HASH_DIM : (j + 1) * HASH_DIM],
                    out_offset=None,
                    in_=tab_d[:],
                    in_offset=IndirectOffsetOnAxis(
                        ap=idx_t[:, pb + j : pb + j + 1], axis=0
                    ),
                )
            eT_ps = ps_t.tile([nblocks * HASH_DIM, P], bf16)
            nc.tensor.transpose(eT_ps[:], embp[:], ident[:])
            eT = embT_p.tile([nblocks * HASH_DIM, P], bf16)
            nc.vector.tensor_copy(eT[:], eT_ps[:])
            o_t = out_p.tile([P, nblocks * MODEL_DIM], bf16, name="o_t")
            for j in range(nblocks):
                for n in range(MODEL_DIM // NFREE):
                    mm = ps_mm.tile([P, NFREE], f32)
                    nc.tensor.matmul(
                        mm[:],
                        lhsT=eT[j * HASH_DIM : (j + 1) * HASH_DIM, :],
                        rhs=projT_s[
                            j * HASH_DIM : (j + 1) * HASH_DIM,
                            n * NFREE : (n + 1) * NFREE,
                        ],
                        start=True,
                        stop=True,
                    )
                    dst = o_t[
                        :, j * MODEL_DIM + n * NFREE : j * MODEL_DIM + (n + 1) * NFREE
                    ]
                    if (pb + j) % 2 == 0:
                        nc.vector.tensor_copy(dst, mm[:])
                    else:
                        nc.scalar.copy(dst, mm[:])
            dst_ap = out_d[pb * P : (pb + nblocks) * P, :].rearrange(
                "(b p) m -> p b m", p=P
            )
            nc.sync.dma_start(
                out=dst_ap, in_=o_t[:].rearrange("p (b m) -> p b m", m=MODEL_DIM)
            )
    nc.compile()
    return nc


def _route_quantile(h):
    """Sort tokens by bucket id and split into equal-count groups."""
    T = h.shape[0]
    order = np.argsort(h, kind="stable")
    offsets = np.asarray([(T * c) // N_CORES for c in range(N_CORES + 1)], np.int64)
    return order, offsets


def _route_owner(h):
    """Fallback: fixed 1/8-range shards, owner routing (unbalanced)."""
    shard_w = -(-NUM_BUCKETS // N_CORES)
    owner = h // shard_w
    order = np.lexsort((h, owner))
    counts = np.bincount(owner, minlength=N_CORES).astype(np.int64)
    offsets = np.zeros(N_CORES + 1, dtype=np.int64)
    np.cumsum(counts, out=offsets[1:])
    bases = [c * shard_w for c in range(N_CORES)]
    return order, offsets, bases


def prepare(input_ids, table, proj_w):
    """Route tokens, build per-core in_maps, fetch/compile the program."""
    B, S = input_ids.shape
    T = B * S
    ids = np.asarray(input_ids, dtype=np.int64)
    prev = np.empty_like(ids)
    prev[:, 0] = 0
    prev[:, 1:] = ids[:, :-1]
    h = ((prev * HASH_MULT + ids) % NUM_BUCKETS).reshape(-1)

    table_bf = np.asarray(table, dtype=np.float32).astype(ml_dtypes.bfloat16)
    projT = np.ascontiguousarray(
        np.asarray(proj_w, dtype=np.float32).T.astype(ml_dtypes.bfloat16)
    )

    # fast path: quantile routing + per-chunk gather windows
    order, offsets = _route_quantile(h)
    h_sorted = h[order]
    counts = np.diff(offsets)
    cap = max(2 * P, int(-(-counts.max() // (2 * P))) * 2 * P)
    K = cap // P
    pre, chunks, _bounds = _plan(K)

    rows_by_core = []
    for c in range(N_CORES):
        padded = np.zeros(cap, dtype=np.int64)
        padded[: counts[c]] = h_sorted[offsets[c] : offsets[c + 1]]
        rows_by_core.append(padded)

    # dg feasibility: each chunk's sorted-row span must fit an int16 window
    use_dg = True
    for c in range(N_CORES):
        for b0, nb in chunks:
            n_real = min(max(0, counts[c] - b0 * P), nb * P)
            if n_real == 0:
                continue
            seg = rows_by_core[c][b0 * P : b0 * P + n_real]
            if seg[-1] - seg[0] >= W:
                use_dg = False

    if use_dg:
        nc = _prog_cache.get(("dg", K))
        if nc is None:
            nc = _build_program(K)
            _prog_cache[("dg", K)] = nc
        in_maps = []
        for c in range(N_CORES):
            rows = rows_by_core[c]
            # pre-gathered transposed leading blocks: [128, pre*128] with
            # the hash dims duplicated on partition halves
            tb = table_bf[np.minimum(rows[: pre * P], NUM_BUCKETS - 1)]
            e0 = tb.T  # [64, pre*P]
            m = {
                "projhi": projT,
                "boot": np.ascontiguousarray(
                    np.hstack([e0[:, : 2 * P], projT, e0[:, 2 * P :]])
                ),
            }
            if chunks:
                wins, cols = [], []
                for b0, nb in chunks:
                    n_real = min(max(0, counts[c] - b0 * P), nb * P)
                    first = int(rows[b0 * P]) if n_real else 0
                    first = min(first, max(NUM_BUCKETS - W, 0))
                    win = table_bf[first : first + W]
                    if win.shape[0] < W:
                        win = np.concatenate(
                            [
                                win,
                                np.zeros(
                                    (W - win.shape[0], HASH_DIM),
                                    dtype=ml_dtypes.bfloat16,
                                ),
                            ]
                        )
                    wins.append(win)
                    rel = np.clip(rows[b0 * P : (b0 + nb) * P] - first, 0, W - 1)
                    cols.append(
                        np.ascontiguousarray(
                            rel.reshape(nb * P // 16, 16).T.astype(np.int16)
                        )
                    )
                tabwin = np.concatenate(wins)  # [n*W, 64]
                # 128-wide with duplicated rows for the transposed gather
                m["table"] = np.ascontiguousarray(
                    np.concatenate([tabwin, tabwin], axis=1)
                )
                idx16 = np.concatenate(cols, axis=1)  # [16, n_idx_cols]
                m["idx16"] = np.ascontiguousarray(np.tile(idx16, (P // 16, 1)))
            in_maps.append(m)
        meta = (T, order, offsets, counts, K)
        return nc, in_maps, meta

    # fallback: owner routing + per-block indirect gathers
    order, offsets, bases = _route_owner(h)
    h_sorted = h[order]
    counts = np.diff(offsets)
    cap = max(2 * P, int(-(-counts.max() // (2 * P))) * 2 * P)
    K = cap // P
    nc = _prog_cache.get(("ind", K))
    if nc is None:
        nc = _build_program_ind(K)
        _prog_cache[("ind", K)] = nc
    in_maps = []
    for c in range(N_CORES):
        loc = h_sorted[offsets[c] : offsets[c + 1]] - bases[c]
        padded = np.zeros(cap, dtype=np.int64)
        padded[: counts[c]] = loc
        lo, hi = bases[c], min(bases[c] + SHARD_CAP, NUM_BUCKETS)
        shard = table_bf[lo:hi]
        if hi - lo < SHARD_CAP:
            shard = np.concatenate(
                [
                    shard,
                    np.zeros(
                        (SHARD_CAP - (hi - lo), HASH_DIM), dtype=ml_dtypes.bfloat16
                    ),
                ]
            )
        in_maps.append(
            {
                "projT": projT,
                "table": np.ascontiguousarray(shard),
                "idx": np.ascontiguousarray(padded.astype(np.int32).reshape(K, P).T),
            }
        )
    meta = (T, order, offsets, counts, K)
    return nc, in_maps, meta


def kernel(input_ids: np.ndarray, table: np.ndarray, proj_w: np.ndarray) -> np.ndarray:
    B, S = input_ids.shape
    nc, in_maps, meta = prepare(input_ids, table, proj_w)
    T, order, offsets, counts, K = meta
    res = run_bass_kernel_spmd(nc, in_maps, list(range(N_CORES)))
    flat = np.empty((T, MODEL_DIM), dtype=np.float32)
    for c in range(N_CORES):
        flat[order[offsets[c] : offsets[c + 1]]] = (
            res.results[c]["out"][: counts[c]].astype(np.float32)
        )
    return flat.reshape(B, S, MODEL_DIM)
